# revision 38
# baseline (speedup 1.0000x reference)
"""GAT forward kernel for Trainium2 (8 NeuronCores, Bass/Tile).

Reference computation (dense form):
    adj = densify(A); Wh = X @ Ws; e = leaky_relu(Wh@a1 + (Wh@a2).T, 0.15)
    att = softmax(where(adj>0, e, -9e15), axis=1); out = elu(att @ Wh)

Sparse form (only ~524K of 16384^2 entries matter; |e| <= ~16 so softmax
needs no max-subtraction):
    w_e = exp(leaky(s_src + t_dst));  out_i = elu(sum_e w_e Wh_dst / sum_e w_e)

Sharding: rows (softmax queries) split 2048/core across 8 cores.

Two-tier edge layout, host pre-gathers X[dst_e] per slot (contiguous bf16
DMA, no device gather):

Tier 1 (first Q=32 edges of every row): slot (b, q, p) holds edge q of row
(b, p).  One matmul per (b, q) chunk (lhsT = XgT chunk, rhs = [Ws@a2|Ws|1])
gives [t_dst, Wh_dst, 1] for 128 rows at once, ROW-partition.  s_src is a
per-partition scalar broadcast, pad slots are masked with -1e30 before exp,
and the softmax aggregation is a plain DVE reduction over q -- no one-hot
matmuls, no PM machinery.

Tier 2 (edges Q..deg of rows with deg > Q, ~7% of slots): classic row-major
cell layout with per-chunk one-hot (is_equal vs srcrel) aggregation matmuls
and the PM cell trick for per-slot s.  Its acc joins tier 1's U in the
epilogue: out = elu((U1+U2) / (Z1+Z2)).

Host prep pads tiers to cross-core uniform chunk counts so all 8 cores run
the same program (SPMD).
"""
import os
import sys

if "/opt/trn_rl_repo" not in sys.path:
    sys.path.insert(0, "/opt/trn_rl_repo")

_ABL = set(os.environ.get("GAT_ABLATE", "").split(","))

from contextlib import ExitStack

import numpy as np

import concourse.bass as bass
import concourse.tile as tile
from concourse import bacc, mybir
from concourse.bass_utils import run_bass_kernel_spmd

N = 16384          # nodes
F = 128            # input features
D = 64             # embedding dim
NCORES = 8
R = N // NCORES    # rows per core (2048)
NB = R // 128      # row blocks per core (16)
Q = 36             # tier-1 slots per row
S1 = NB * Q * 128  # tier-1 slots per core (65536)
CELL = 8           # tier-2 slots per cell (one src row per cell)
SUB = 11           # A1 chunks per PSUM sub-batch (11*65*4B < 2 banks)
TPAD = -400.0      # pad-slot sentinel: t_pad ~ TPAD so exp(leaky) ~ 0
dt = mybir.dt


# ---------------------------------------------------------------- host prep
def _prep_edges(A):
    """Dedup edges; tier-1 = first Q edges per row in (block, q, row) slot
    order with a -inf pad mask; tier-2 = remaining edges in row-major
    CELL-padded layout, padded to cross-core uniform chunk counts Kb2."""
    src_all = np.asarray(A[0], dtype=np.int64)
    dst_all = np.asarray(A[1], dtype=np.int64)
    keys = np.unique(src_all * N + dst_all)     # dedup + sort by (src, dst)
    src = (keys // N).astype(np.int32)
    dst = (keys % N).astype(np.int32)
    E = len(dst)

    deg = np.bincount(src, minlength=N)
    assert deg.min() > 0, (
        "empty rows present; this kernel assumes every row has >=1 edge"
    )
    row_start = np.concatenate([[0], np.cumsum(deg)])

    # ---- tier 1: dsti1 [NCORES, NB, Q, 128] + pad mask (pads get the
    # sentinel X column so t_pad ~ TPAD and w vanishes without a mask op)
    rows = np.arange(N).reshape(NCORES, NB, 128)
    qs = np.arange(Q)
    pos = row_start[rows][..., None] + qs                 # [C, NB, 128, Q]
    valid = qs[None, None, None, :] < deg[rows][..., None]
    dsti1 = np.where(valid, dst[np.minimum(pos, E - 1)], 0)
    dsti1 = dsti1.transpose(0, 1, 3, 2).astype(np.int32)  # [C, NB, Q, 128]
    pad1 = (~valid).transpose(0, 1, 3, 2).reshape(NCORES, S1)

    # ---- tier 2: excess edges, row-major cells, block buckets
    exc = np.maximum(deg - Q, 0)
    excc = ((exc + CELL - 1) // CELL) * CELL
    slots_cb = excc.reshape(NCORES, NB, 128).sum(axis=2)   # [C, NB]
    Kb2 = np.maximum((slots_cb.max(axis=0) + 127) // 128, 1)   # [NB]
    S2 = int(Kb2.sum()) * 128
    offs2 = np.concatenate([[0], np.cumsum(Kb2)]) * 128
    cells_cb = slots_cb // CELL
    ncell2 = [int(k) * (128 // CELL) for k in Kb2]
    ncellp2 = [((n + 127) // 128) * 128 for n in ncell2]
    cell_offs2 = np.concatenate([[0], np.cumsum(ncellp2)])
    SC2 = int(cell_offs2[-1])

    dsti2 = np.zeros((NCORES, S2), np.int32)
    srel2 = np.full((NCORES, S2), -1.0, np.float32)
    cellsrc2 = np.zeros((NCORES, SC2), np.int16)
    for c in range(NCORES):
        for b in range(NB):
            pos2 = offs2[b]
            for p in range(128):
                r = (c * NB + b) * 128 + p
                d = int(exc[r])
                if d == 0:
                    continue
                lo = row_start[r] + Q
                dsti2[c, pos2:pos2 + d] = dst[lo:lo + d]
                srel2[c, pos2:pos2 + d] = float(p)
                ncw = int(excc[r])
                cbase = cell_offs2[b] + (pos2 - offs2[b]) // CELL
                cellsrc2[c, cbase:cbase + ncw // CELL] = r
                pos2 += ncw
            assert pos2 <= offs2[b + 1]

    import ml_dtypes
    NMTOT = sum(n // 128 for n in ncellp2)
    cores = []
    for c in range(NCORES):
        PMs = np.zeros((128, NMTOT, 128), ml_dtypes.bfloat16)
        g = 0
        for b in range(NB):
            base = (c * NB + b) * 128
            for m in range(ncellp2[b] // 128):
                cs = cellsrc2[c, cell_offs2[b] + m * 128:
                              cell_offs2[b] + (m + 1) * 128]
                rel = cs.astype(np.int64) - base
                vv = (rel >= 0) & (rel < 128)
                PMs[rel[vv], g, np.arange(128)[vv]] = 1.0
                g += 1
        assert g == NMTOT
        cores.append({
            "dsti": np.concatenate([dsti1[c].reshape(-1), dsti2[c]]),
            "pad1": pad1[c],                                     # [S1] bool
            "srel2": srel2[c].reshape(S2 // 128, 128).T.copy(),  # [128, S2/128]
            "PMs": PMs,
        })
    return cores, [int(k) for k in Kb2], S2, [int(x) for x in ncellp2]


# ---------------------------------------------------------------- device IR
def _build(Kb2, S2, ncellp2):
    SC2 = sum(ncellp2)
    NMTOT = SC2 // 128
    ST = S1 + S2
    nc = bacc.Bacc("TRN2", target_bir_lowering=False, debug=False,
                   enable_asserts=False, num_devices=NCORES,
                   num_swdge_queues=4)
    XgT_d = nc.dram_tensor("XgT", [F, ST], dt.bfloat16, kind="ExternalInput").ap()
    XTown_d = nc.dram_tensor("XTown", [F, R], dt.bfloat16, kind="ExternalInput").ap()
    Ws_d = nc.dram_tensor("Ws", [F, D], dt.float32, kind="ExternalInput").ap()
    WsT_d = nc.dram_tensor("WsT", [D, F], dt.float32, kind="ExternalInput").ap()
    apair_d = nc.dram_tensor("apair", [D, 2], dt.float32, kind="ExternalInput").ap()
    PMs_d = nc.dram_tensor("PMs", [128, NMTOT, 128], dt.bfloat16, kind="ExternalInput").ap()
    srel2_d = nc.dram_tensor("srel2", [128, S2 // 128], dt.float32, kind="ExternalInput").ap()
    sel16_d = nc.dram_tensor("sel16", [128, CELL], dt.float32, kind="ExternalInput").ap()
    E16_d = nc.dram_tensor("E16", [128, 128], dt.float32, kind="ExternalInput").ap()
    iotaf_d = nc.dram_tensor("iotaf", [128, 128], dt.float32, kind="ExternalInput").ap()
    out_d = nc.dram_tensor("out", [R, D], dt.float32, kind="ExternalOutput").ap()

    with tile.TileContext(nc) as tc, ExitStack() as ctx:
        cpool = ctx.enter_context(tc.tile_pool(name="const", bufs=1))
        xgpool = ctx.enter_context(tc.tile_pool(name="xg", bufs=3))
        x2pool = ctx.enter_context(tc.tile_pool(name="x2", bufs=3))
        whpool = ctx.enter_context(tc.tile_pool(name="wh", bufs=3))
        Gpool = ctx.enter_context(tc.tile_pool(name="G", bufs=3))
        ohpool = ctx.enter_context(tc.tile_pool(name="oh", bufs=3))
        wpool = ctx.enter_context(tc.tile_pool(name="w", bufs=3))
        epool = ctx.enter_context(tc.tile_pool(name="ep", bufs=3))
        # PSUM budget (8 banks): wt 2x1, acc2 2x1, sc 1, se 2x1
        ps_wt = ctx.enter_context(tc.tile_pool(name="ps_wt", bufs=2, space="PSUM"))
        ps_wt2 = ctx.enter_context(tc.tile_pool(name="ps_wt2", bufs=1, space="PSUM"))
        ps_acc = ctx.enter_context(tc.tile_pool(name="ps_acc", bufs=2, space="PSUM"))
        ps_se = ctx.enter_context(tc.tile_pool(name="ps_se", bufs=1, space="PSUM"))

        # ---- constants
        iota_f = cpool.tile([128, 128], dt.float32)
        nc.sync.dma_start(iota_f[:], iotaf_d)
        ws_t = cpool.tile([F, D], dt.float32)
        nc.sync.dma_start(ws_t[:], Ws_d)
        wsT_t = cpool.tile([D, F], dt.float32)
        nc.sync.dma_start(wsT_t[:], WsT_d)
        apair_t = cpool.tile([D, 2], dt.float32)
        nc.sync.dma_start(apair_t[:], apair_d)
        sel16_t = cpool.tile([128, CELL], dt.float32)
        nc.sync.dma_start(sel16_t[:], sel16_d)
        E16_t = cpool.tile([128, 128], dt.float32)
        nc.sync.dma_start(E16_t[:], E16_d)
        PMs_t = cpool.tile([128, NMTOT, 128], dt.bfloat16)
        nc.scalar.dma_start(PMs_t[:], PMs_d)
        srel2_t = cpool.tile([128, S2 // 128], dt.float32)
        nc.scalar.dma_start(srel2_t[:], srel2_d)
        xtown_t = cpool.tile([F, NB, 128], dt.bfloat16)
        nc.scalar.dma_start(xtown_t[:], XTown_d.rearrange("f (b p) -> f b p", p=128))

        # Wse = [Ws@a2 | Ws] bf16: one 65-col rhs so each A1 matmul yields
        # [t_dst, Wh_dst] per slot.  Wsa1 bf16 for the s matmuls.
        wsa_ps = ps_se.tile([128, 2], dt.float32, space="PSUM", tag="se")
        nc.tensor.matmul(wsa_ps[:], lhsT=wsT_t[:], rhs=apair_t[:],
                         start=True, stop=True)
        Wse = cpool.tile([F, 1 + D], dt.bfloat16)
        nc.vector.tensor_copy(Wse[:, 0:1], wsa_ps[:, 0:1])
        nc.vector.tensor_copy(Wse[:, 1:1 + D], ws_t[:])
        wsa1_t = cpool.tile([F, 1], dt.bfloat16)
        nc.vector.tensor_copy(wsa1_t[:], wsa_ps[:, 1:2])

        # ---- s for own rows: s[r] = X[r] @ Ws @ a1, per block -> [128, NB]
        s_ps = ps_se.tile([128, NB], dt.float32, space="PSUM", tag="se")
        for b in range(NB):
            nc.tensor.matmul(s_ps[:, b:b + 1], lhsT=xtown_t[:, b, :],
                             rhs=wsa1_t[:], start=True, stop=True)
        sloc = cpool.tile([128, NB], dt.float32)
        nc.vector.tensor_copy(sloc[:], s_ps[:])
        # hi/lo bf16 split so the tier-2 PM matmuls stay near-f32 exact
        sloc_hl = cpool.tile([128, NB, 2], dt.bfloat16)
        nc.vector.tensor_copy(sloc_hl[:, :, 0], s_ps[:])
        nc.vector.tensor_sub(sloc_hl[:, :, 1], s_ps[:], sloc_hl[:, :, 0])

        # ---- main loop: per 128-row block
        sl2 = [0]
        for b in range(NB):
            sl2.append(sl2[-1] + Kb2[b])
        XgT1_v = XgT_d[:, 0:S1].rearrange("f (b q p) -> f b q p", q=Q, p=128)
        out_v = out_d.rearrange("(b p) d -> p b d", p=128)   # [128, NB, D]
        outstage = cpool.tile([128, NB, D], dt.float32)
        gsp = 0          # global PM span index
        for b in range(NB):
            K2 = Kb2[b]
            nm2 = ncellp2[b] // 128
            # ================= tier 1 =================
            xg1 = xgpool.tile([128, Q, 128], dt.bfloat16)
            nc.sync.dma_start(xg1[:], XgT1_v[:, b])
            e_t = wpool.tile([128, Q], dt.float32, tag="e")
            whp = whpool.tile([128, 1 + D, Q], dt.bfloat16)   # [Wh|1][d, q]
            nc.vector.memset(whp[:, D, :], 1.0)
            # sub-batches through a 2-bank PSUM tile: 7 chunks of 65 per bank
            # (matmul outputs must stay inside one 512-f32 bank)
            c0 = 0
            subs = []
            r = Q
            while r:
                n = min(14, r)
                subs.append((2, n // 2) if n > 7 else (1, n))
                r -= n
            for nb, ns in subs:
                n = nb * ns
                wt_ps = ps_wt.tile([128, 2, 512], dt.float32,
                                   space="PSUM", tag="wt")
                for j in range(n):
                    nc.tensor.matmul(
                        wt_ps[:, j // ns, (j % ns) * 65:(j % ns) * 65 + 65],
                        lhsT=xg1[:, c0 + j, :],
                        rhs=Wse[:], start=True, stop=True)
                wt_v = wt_ps[:, 0:nb, 0:ns * 65].rearrange(
                    "p b (s d) -> p b s d", d=65)
                nc.vector.tensor_scalar_add(
                    e_t[:, c0:c0 + n].rearrange("p (b s) -> p b s", s=ns),
                    wt_v[:, :, :, 0], sloc[:, b:b + 1])
                nc.scalar.activation(
                    whp[:, 0:D, c0:c0 + n].rearrange("p d (b s) -> p d b s", s=ns),
                    wt_v[:, :, :, 1:].rearrange("p b s d -> p d b s"),
                    mybir.ActivationFunctionType.Copy)
                c0 += n
            # w = exp(leaky(e)), e = s + t computed during eviction above
            lk = wpool.tile([128, Q], dt.float32, tag="lk")
            nc.vector.scalar_tensor_tensor(
                out=lk[:], in0=e_t[:], scalar=0.15, op0=mybir.AluOpType.mult,
                in1=e_t[:], op1=mybir.AluOpType.max)
            w_t = wpool.tile([128, Q], dt.bfloat16, tag="wt")
            nc.scalar.activation(w_t[:], lk[:], mybir.ActivationFunctionType.Exp)
            # G[d, q] = w[q] * [Wh, 1][d, q]; U[d] = sum_q G[d, q]
            G = Gpool.tile([128, 1 + D, Q], dt.bfloat16)
            nc.vector.tensor_mul(G[:], whp[:],
                                 w_t[:, None, :].to_broadcast([128, 1 + D, Q]))
            # two pairwise bf16 folds quarter the f32-out reduce's 1x work
            H = Gpool.tile([128, 1 + D, Q // 2], dt.bfloat16, tag="H")
            nc.vector.tensor_add(H[:], G[:, :, 0:Q // 2], G[:, :, Q // 2:Q])
            H2 = Gpool.tile([128, 1 + D, Q // 4], dt.bfloat16, tag="H2")
            nc.vector.tensor_add(H2[:], H[:, :, 0:Q // 4], H[:, :, Q // 4:Q // 2])
            U_t = epool.tile([128, 1 + D], dt.float32, tag="U")
            nc.vector.reduce_sum(U_t[:], H2[:], axis=mybir.AxisListType.X)
            # ================= tier 2 =================
            lo2 = S1 + sl2[b] * 128
            xg2 = x2pool.tile([128, K2, 128], dt.bfloat16)
            nc.sync.dma_start(
                xg2[:], XgT_d[:, lo2:lo2 + K2 * 128]
                .rearrange("f (k p) -> f k p", p=128))
            oht2 = ohpool.tile([128, K2, 128], dt.bfloat16)
            nc.vector.tensor_tensor(
                out=oht2[:],
                in0=iota_f[:, None, :].to_broadcast([128, K2, 128]),
                in1=srel2_t[:, sl2[b]:sl2[b] + K2, None]
                    .to_broadcast([128, K2, 128]),
                op=mybir.AluOpType.is_equal)
            t2_f = wpool.tile([128, K2], dt.float32, tag="t2")
            wh2 = whpool.tile([128, K2, 1 + D], dt.bfloat16)
            for c0 in range(0, K2, 7):
                c1 = min(c0 + 7, K2)
                wt_ps = ps_wt2.tile([128, 512], dt.float32,
                                    space="PSUM", tag="wt2")
                for j in range(c1 - c0):
                    nc.tensor.matmul(wt_ps[:, j * 65:j * 65 + 65],
                                     lhsT=xg2[:, c0 + j, :],
                                     rhs=Wse[:], start=True, stop=True)
                wt2_v = wt_ps[:, 0:(c1 - c0) * 65].rearrange(
                    "p (s d) -> p s d", d=65)
                nc.vector.tensor_copy(t2_f[:, c0:c1], wt2_v[:, :, 0])
                nc.scalar.activation(
                    wh2[:, c0:c1, 0:D], wt2_v[:, :, 1:],
                    mybir.ActivationFunctionType.Copy)
            # s per cell via PM one-hot matmuls, then expand cells -> slots
            see_ps = ps_se.tile([128, nm2 * (CELL + 1)], dt.float32,
                                space="PSUM", tag="se")
            sc_ps = see_ps[:, nm2 * CELL:]
            se_ps = see_ps[:, 0:nm2 * CELL]
            for m in range(nm2):
                nc.tensor.matmul(sc_ps[:, m:m + 1], lhsT=PMs_t[:, gsp + m, :],
                                 rhs=sloc_hl[:, b, 0:1], start=True, stop=False)
                nc.tensor.matmul(sc_ps[:, m:m + 1], lhsT=PMs_t[:, gsp + m, :],
                                 rhs=sloc_hl[:, b, 1:2], start=False, stop=True)
            for m in range(nm2):
                rhsm = wpool.tile([128, CELL], dt.float32, tag="rhsm")
                nc.vector.tensor_mul(
                    rhsm[:], sel16_t[:],
                    sc_ps[:, m:m + 1].to_broadcast([128, CELL]))
                nc.tensor.matmul(se_ps[:, m * CELL:(m + 1) * CELL],
                                 lhsT=E16_t[:], rhs=rhsm[:],
                                 start=True, stop=True)
            e2 = wpool.tile([128, K2], dt.float32, tag="e2")
            nc.vector.tensor_add(e2[:], se_ps[:, 0:K2], t2_f[:])
            lk2 = wpool.tile([128, K2], dt.float32, tag="lk2")
            nc.vector.scalar_tensor_tensor(
                out=lk2[:], in0=e2[:], scalar=0.15, op0=mybir.AluOpType.mult,
                in1=e2[:], op1=mybir.AluOpType.max)
            w2 = wpool.tile([128, K2], dt.bfloat16, tag="w2")
            nc.scalar.activation(w2[:], lk2[:], mybir.ActivationFunctionType.Exp)
            G2 = Gpool.tile([128, K2, 1 + D], dt.bfloat16)
            nc.vector.tensor_mul(G2[:, :, 0:D], wh2[:, :, 0:D],
                                 w2[:, :, None].to_broadcast([128, K2, D]))
            nc.vector.tensor_copy(G2[:, :, D], w2[:])
            acc2 = ps_acc.tile([128, 1 + D], dt.float32, space="PSUM", tag="acc")
            for c in range(K2):
                nc.tensor.matmul(acc2[:], lhsT=oht2[:, c, :], rhs=G2[:, c, :],
                                 start=(c == 0), stop=(c == K2 - 1))
            # ============ epilogue (block pairs): out = elu(U/Z) ============
            if b % 4 == 0:
                Utb = epool.tile([128, 4, 1 + D], dt.float32, tag="Ut")
            nc.vector.tensor_add(Utb[:, b % 4, :], U_t[:], acc2[:])
            if b % 4 == 3:
                zg = epool.tile([128, 4], dt.float32, tag="zg")
                nc.vector.tensor_scalar_max(zg[:], Utb[:, :, D], 1e-30)
                zr = epool.tile([128, 4], dt.float32, tag="zr")
                nc.vector.reciprocal(zr[:], zg[:])
                x = epool.tile([128, 4, D], dt.float32, tag="x")
                nc.vector.tensor_mul(x[:], Utb[:, :, 0:D],
                                     zr[:, :, None].to_broadcast([128, 4, D]))
                mn = epool.tile([128, 4, D], dt.float32, tag="mn")
                nc.vector.tensor_scalar_min(mn[:], x[:], 0.0)
                em = epool.tile([128, 4, D], dt.float32, tag="em")
                nc.scalar.activation(em[:], mn[:],
                                     mybir.ActivationFunctionType.Exp)
                rl = epool.tile([128, 4, D], dt.float32, tag="rl")
                nc.vector.tensor_scalar_max(rl[:], x[:], 0.0)
                nc.vector.scalar_tensor_tensor(
                    out=outstage[:, b - 3:b + 1, :], in0=em[:], scalar=-1.0,
                    op0=mybir.AluOpType.add, in1=rl[:], op1=mybir.AluOpType.add)
            if b == NB // 2 - 1:
                nc.sync.dma_start(out_v[:, 0:NB // 2, :],
                                  outstage[:, 0:NB // 2, :])
            gsp += nm2

        nc.sync.dma_start(out_v[:, NB // 2:, :], outstage[:, NB // 2:, :])
    nc.compile()
    return nc


_cache = {}


def _get_program(Kb2, S2, ncellp2):
    key = (tuple(Kb2), S2, tuple(ncellp2), tuple(sorted(_ABL)))
    if key not in _cache:
        _cache[key] = _build(Kb2, S2, ncellp2)
    return _cache[key]


def make_in_maps(A, X, Ws, a):
    """Host-side sharding: returns (nc, in_maps)."""
    import ml_dtypes
    X = np.asarray(X, dtype=np.float32)
    Ws = np.ascontiguousarray(np.asarray(Ws, dtype=np.float32))
    a = np.asarray(a, dtype=np.float32).reshape(2 * D)
    Xbf = X.astype(ml_dtypes.bfloat16)
    WsT = np.ascontiguousarray(Ws.T)
    apair = np.stack([a[D:], a[:D]], axis=1).astype(np.float32)  # [D, 2] = [a2|a1]
    q = np.arange(128)
    CPC = 128 // CELL
    sel16 = (q[:, None] // CPC == np.arange(CELL)[None, :]).astype(np.float32)
    E16 = (q[:, None] % CPC == q[None, :] // CELL).astype(np.float32)
    iotaf = np.tile(np.arange(128, dtype=np.float32)[None, :], (128, 1))
    cores, Kb2, S2, ncellp2 = _prep_edges(A)
    nc = _get_program(Kb2, S2, ncellp2)
    # pad sentinel column: v . (Ws@a2) = TPAD so pad slots get t ~ TPAD and
    # w = exp(leaky(s+t)) ~ e^-58 ~ 0 with no mask op on device
    wsa2 = Ws @ a[D:]
    nrm = float((wsa2 ** 2).sum())
    assert nrm > 1e-8, "degenerate Ws@a2; sentinel padding invalid"
    vpad = (TPAD / nrm) * wsa2
    vpad_bf = vpad.astype(ml_dtypes.bfloat16)
    in_maps = []
    for c in range(NCORES):
        ci = cores[c]
        Xg = Xbf[ci["dsti"]]                                    # [S1+S2, F]
        Xg[:S1][ci["pad1"]] = vpad_bf
        XgT = np.ascontiguousarray(Xg.T)                        # [F, S1+S2]
        XTown = np.ascontiguousarray(Xbf[c * R:(c + 1) * R].T)  # [F, R]
        in_maps.append({
            "XgT": XgT, "XTown": XTown, "Ws": Ws, "WsT": WsT,
            "apair": apair,
            "sel16": sel16, "E16": E16, "iotaf": iotaf,
            "srel2": ci["srel2"], "PMs": ci["PMs"],
        })
    return nc, in_maps


def kernel(A, X, Ws, a):
    nc, in_maps = make_in_maps(A, X, Ws, a)
    res = run_bass_kernel_spmd(nc, in_maps, core_ids=list(range(NCORES)),
                               trace=False)
    return np.concatenate([r["out"] for r in res.results], axis=0)
